# revision 10
# baseline (speedup 1.0000x reference)
"""HOPEBlock Trainium2 kernel — 8-core zero-collective sequence parallel.

Sharding: core c = (g, r), g = c // 4 (batch element), r = c % 4 (query-token
chunk). Each core runs the whole block end-to-end for its 512 query tokens:
k/v for all 2048 tokens of its batch element are computed locally (duplicated
across the 4 cores of a group) so that no collective is needed — collectives
dominate wall time in this environment by ~3 orders of magnitude.

All activations are kept feature-major on-chip: [feature_dim, tokens].
Matmul inputs are bf16 (fp32 PSUM accumulation); biases and RMS statistics
stay fp32.
"""

import numpy as np
import ml_dtypes
from contextlib import ExitStack

import concourse.bass as bass
import concourse.tile as tile
from concourse import bacc, mybir, library_config
from concourse.bass_utils import run_bass_kernel_spmd

F32 = mybir.dt.float32
BF16 = mybir.dt.bfloat16
AF = mybir.ActivationFunctionType
OP = mybir.AluOpType

B, S, H = 2, 2048, 1024
HEADS, HD = 16, 64
INNER = 4 * H
NCORES = 8
GSZ = 4                     # cores per batch element
Q = S // GSZ                # 512 query tokens per core
NP = HEADS // 2             # 8 head pairs
SC = S // 128               # 16 key chunks of 128
ROPE_THETA = 10000.0
RMS_EPS = 1.1920929e-07

NP_BF16 = ml_dtypes.bfloat16

_cached = {}


def build_program(reps=1, no_coll=False):
    key = ("nc", reps)
    if key in _cached:
        return _cached[key]
    nc = bacc.Bacc("TRN2", target_bir_lowering=False, debug=False,
                   num_devices=NCORES)

    def din(name, shape, dt=BF16):
        return nc.dram_tensor(name, shape, dt, kind="ExternalInput")

    # xt/cosf/sinf columns are token-permuted per core: own 512 query tokens
    # first, the rest after (softmax is order-invariant over keys).
    xt = din("xt", [H, S])                 # x[g].T  (feature-major) bf16
    xc = din("xc", [H, Q], F32)            # own-chunk fp32 (residual)
    qkt = din("qkt", [H, 16 * 128])        # q,k weightsT: 8 q-pair cols then 8 k-pair cols
    vwt = din("vwt", [H, H])               # v weightsT, natural head-major cols
    owt = din("owt", [H, H])               # out_w.T
    fc1t = din("fc1t", [H, INNER])
    fc1b = din("fc1b", [128, 32], F32)
    fc2t = din("fc2t", [INNER, H])
    fc2b = din("fc2b", [128, 8], F32)
    updt = din("updt", [H, H])
    updb = din("updb", [128, 8], F32)
    sct = din("sct", [H, H])
    scb = din("scb", [128, 8], F32)
    normw = din("normw", [128, 8], F32)
    cosf = din("cosf", [128, S])
    sinf = din("sinf", [128, S])
    out = nc.dram_tensor("out", [H, Q], F32, kind="ExternalOutput")

    with tile.TileContext(nc) as tc:
        for _rep in range(reps):
            _emit_iter(nc, tc, xt, xc, qkt, vwt, owt, fc1t, fc1b,
                       fc2t, fc2b, updt, updb, sct, scb, normw,
                       cosf, sinf, out)

    nc.compile()
    _cached[key] = nc
    return nc


def _emit_iter(nc, tc, xt, xc, qkt, vwt, owt, fc1t, fc1b, fc2t, fc2b,
               updt, updb, sct, scb, normw, cosf, sinf, out):
    with ExitStack() as ctx:
        persist = ctx.enter_context(tc.tile_pool(name="persist", bufs=1))
        xc_sb = persist.tile([128, 8, Q], F32, tag="xc")
        nc.sync.dma_start(xc_sb[:], xc.ap().rearrange("(c p) t -> p c t", p=128))
        fc1b_sb = persist.tile([128, 32], F32, tag="fc1b")
        nc.sync.dma_start(fc1b_sb[:], fc1b.ap())
        fc2b_sb = persist.tile([128, 8], F32, tag="fc2b")
        nc.sync.dma_start(fc2b_sb[:], fc2b.ap())
        updb_sb = persist.tile([128, 8], F32, tag="updb")
        nc.sync.dma_start(updb_sb[:], updb.ap())
        scb_sb = persist.tile([128, 8], F32, tag="scb")
        nc.sync.dma_start(scb_sb[:], scb.ap())
        normw_sb = persist.tile([128, 8], F32, tag="normw")
        nc.sync.dma_start(normw_sb[:], normw.ap())
        denom_sb = persist.tile([16, Q], F32, tag="denom")
        recip_sb = persist.tile([16, Q], F32, tag="recip")
        ones1_sb = persist.tile([128, 1], F32, tag="ones1")
        nc.vector.memset(ones1_sb[:], 1.0)
        sgn_sb = persist.tile([128, 1], F32, tag="sgn")  # -1 on e-blocks, +1 on o-blocks
        for blk in range(4):
            nc.vector.memset(sgn_sb[32 * blk:32 * (blk + 1), :],
                             -1.0 if blk % 2 == 0 else 1.0)
        eps_sb = persist.tile([1, 1], F32, tag="eps")
        nc.vector.memset(eps_sb[:], RMS_EPS)

        # h lives phase D -> E (attention residual, bf16 MLP input)
        hpool = ctx.enter_context(tc.tile_pool(name="hpool", bufs=1))
        h_sb = hpool.tile([128, 8, Q], BF16, tag="h")
        # mixed lives phase E -> F
        mpool = ctx.enter_context(tc.tile_pool(name="mpool", bufs=1))
        mixed_sb = mpool.tile([128, 8, Q], F32, tag="mixed")

        with tc.tile_pool(name="cpool", bufs=1) as cpool:
            q_sb = cpool.tile([128, NP, Q], BF16, tag="q")
            k_sb = cpool.tile([128, NP, S], BF16, tag="k")
            vt_sb = cpool.tile([128, SC, HEADS * 65], BF16, tag="vt")
            on_sb = cpool.tile([128, NP, Q], BF16, tag="on")  # normalized attn out

            # ---------------- Phase A: QKV projections ----------------
            with tc.tile_pool(name="xpool", bufs=1) as xpool:
                x_sb = xpool.tile([128, 8, S], BF16, tag="x")
                nc.sync.dma_start(x_sb[:], xt.ap().rearrange("(c p) t -> p c t", p=128))

                # A0: v for all tokens; ones columns of vt (col 64 per 65-block)
                with tc.tile_pool(name="a0pool", bufs=1) as a0pool, \
                     tc.tile_pool(name="vpsum", bufs=2, space="PSUM") as vpsum:
                    vwt_sb = a0pool.tile([128, 8, H], BF16, tag="vwt")
                    nc.sync.dma_start(vwt_sb[:], vwt.ap().rearrange("(c p) m -> p c m", p=128))
                    vt_v = vt_sb[:].rearrange("p s (h c) -> p s h c", c=65)
                    nc.vector.memset(vt_v[:, :, :, 64], 1.0)
                    for s in range(SC):
                        ps = vpsum.tile([128, H], F32, tag="vps")
                        for hf in range(2):
                            cs = slice(hf * 512, (hf + 1) * 512)
                            for f in range(8):
                                nc.tensor.matmul(
                                    ps[:, cs],
                                    x_sb[:, f, s * 128:(s + 1) * 128],
                                    vwt_sb[:, f, cs],
                                    start=(f == 0), stop=(f == 7))
                        nc.vector.tensor_copy(
                            vt_v[:, s, :, 0:64],
                            ps[:].rearrange("p (h d) -> p h d", d=64))

                # A1: q (own tokens = cols 0:Q) and k (all tokens)
                with tc.tile_pool(name="a1pool", bufs=1) as a1pool, \
                     tc.tile_pool(name="apsum", bufs=3, space="PSUM") as apsum:
                    qkt_sb = a1pool.tile([128, 8, 16 * 128], BF16, tag="qkt")
                    nc.sync.dma_start(qkt_sb[:], qkt.ap().rearrange("(c p) m -> p c m", p=128))
                    for j in range(NP):
                        ps = apsum.tile([128, Q], F32, tag="qkps", name=f"qps{j}")
                        for f in range(8):
                            nc.tensor.matmul(
                                ps[:],
                                qkt_sb[:, f, j * 128:(j + 1) * 128],
                                x_sb[:, f, 0:Q],
                                start=(f == 0), stop=(f == 7))
                        nc.vector.tensor_copy(q_sb[:, j, :], ps[:])
                    for j in range(NP):
                        for t in range(4):
                            ps = apsum.tile([128, 512], F32, tag="qkps",
                                            name=f"kps{j}_{t}")
                            for f in range(8):
                                nc.tensor.matmul(
                                    ps[:],
                                    qkt_sb[:, f, (NP + j) * 128:(NP + j + 1) * 128],
                                    x_sb[:, f, t * 512:(t + 1) * 512],
                                    start=(f == 0), stop=(f == 7))
                            nc.vector.tensor_copy(k_sb[:, j, t * 512:(t + 1) * 512], ps[:])

            # ---------------- Phase B: RoPE on q, k ----------------
            # row blocks per pair tile: [hA-e(32) hA-o(32) hB-e(32) hB-o(32)]
            # q' = A + sgn * blockswap(B),  A = q*cos, B = q*sin
            with tc.tile_pool(name="rpool", bufs=1) as rpool, \
                 tc.tile_pool(name="rwk", bufs=2) as rwk:
                cos_sb = rpool.tile([128, S], BF16, tag="cos")
                nc.sync.dma_start(cos_sb[:], cosf.ap())
                sin_sb = rpool.tile([128, S], BF16, tag="sin")
                nc.sync.dma_start(sin_sb[:], sinf.ap())
                for tens, w in ((q_sb, Q), (k_sb, S)):
                    for j in range(NP):
                        a_t = rwk.tile([128, w], BF16, tag=f"ropeA{w}",
                                       name=f"rA{w}_{j}")
                        b_t = rwk.tile([128, w], BF16, tag=f"ropeB{w}",
                                       name=f"rB{w}_{j}")
                        bs_t = rwk.tile([128, w], BF16, tag=f"ropeBs{w}",
                                        name=f"rBs{w}_{j}")
                        nc.vector.tensor_tensor(a_t[:], tens[:, j, :],
                                                cos_sb[:, 0:w], OP.mult)
                        nc.vector.tensor_tensor(b_t[:], tens[:, j, :],
                                                sin_sb[:, 0:w], OP.mult)
                        for blk in range(4):  # swap e<->o 32-row blocks via DMA
                            src = blk + 1 if blk % 2 == 0 else blk - 1
                            nc.sync.dma_start(
                                bs_t[32 * blk:32 * (blk + 1), :],
                                b_t[32 * src:32 * (src + 1), :])
                        nc.vector.scalar_tensor_tensor(
                            tens[:, j, :], bs_t[:], sgn_sb[:, 0:1], a_t[:],
                            OP.mult, OP.add)

            # ---------------- Phase C: attention ----------------
            with tc.tile_pool(name="spsum", bufs=2, space="PSUM") as spsum, \
                 tc.tile_pool(name="avpsum", bufs=4, space="PSUM") as avpsum, \
                 tc.tile_pool(name="epool", bufs=3) as epool, \
                 tc.tile_pool(name="dnpool", bufs=3) as dnpool:
                for j in range(NP):        # head pair (heads 2j, 2j+1)
                    av = [avpsum.tile([65, Q], F32, tag="av", name=f"av{j}_{i}")
                          for i in range(2)]
                    for s in range(SC):
                        ss = slice(s * 128, (s + 1) * 128)
                        sco = spsum.tile([128, 1024], F32, tag="sco")
                        nc.tensor.matmul(
                            sco[:, 0:512],
                            k_sb[0:64, j, ss], q_sb[0:64, j, :],
                            start=True, stop=True, tile_position=(0, 0))
                        nc.tensor.matmul(
                            sco[:, 512:1024],
                            k_sb[64:128, j, ss], q_sb[64:128, j, :],
                            start=True, stop=True, tile_position=(64, 0))
                        e_t = epool.tile([128, 1024], BF16, tag="exp")
                        nc.scalar.activation(e_t[:], sco[:], AF.Exp)
                        for hl in range(2):  # head 2j + hl
                            nc.tensor.matmul(
                                av[hl][:],
                                vt_sb[:, s, (2 * j + hl) * 65:(2 * j + hl) * 65 + 65],
                                e_t[:, hl * 512:(hl + 1) * 512],
                                start=(s == 0), stop=(s == SC - 1))
                    for hl in range(2):
                        head = 2 * j + hl
                        if hl == 0:
                            nc.vector.tensor_copy(on_sb[0:64, j, :], av[0][0:64, :])
                        else:
                            otmp = dnpool.tile([64, Q], BF16, tag="otmp",
                                               name=f"ot{j}")
                            nc.vector.tensor_copy(otmp[:], av[1][0:64, :])
                            nc.sync.dma_start(on_sb[64:128, j, :], otmp[:])
                        dtmp = dnpool.tile([128, Q], F32, tag="dtmp",
                                           name=f"dt{j}_{hl}")
                        nc.vector.tensor_copy(dtmp[64:65, :], av[hl][64:65, :])
                        nc.sync.dma_start(denom_sb[head:head + 1, :], dtmp[64:65, :])

                # normalize: out <- out * (1/denom) broadcast over the 64 dims
                nc.vector.reciprocal(recip_sb[:], denom_sb[:])
                with tc.tile_pool(name="bcpool", bufs=2) as bcpool:
                    for head in range(HEADS):
                        rtmp = bcpool.tile([1, Q], F32, tag="rtmp",
                                           name=f"rt{head}")
                        nc.sync.dma_start(rtmp[:], recip_sb[head:head + 1, :])
                        bc = bcpool.tile([128, Q], F32, tag="bc",
                                         name=f"bc{head}")
                        nc.gpsimd.partition_broadcast(bc[:], rtmp[:])
                        lo = (head % 2) * 64
                        sl = on_sb[lo:lo + 64, head // 2, :]
                        nc.vector.tensor_tensor(sl, sl, bc[lo:lo + 64, :], OP.mult)

            # ---------------- Phase D: out-proj + residual h ----------------
            with tc.tile_pool(name="dwpool", bufs=1) as dwpool, \
                 tc.tile_pool(name="dpsum", bufs=3, space="PSUM") as dpsum:
                owt_sb = dwpool.tile([128, 8, H], BF16, tag="owt")
                nc.sync.dma_start(owt_sb[:], owt.ap().rearrange("(c p) o -> p c o", p=128))
                for oc in range(8):
                    ps = dpsum.tile([128, Q], F32, tag="aops")
                    for f in range(8):
                        nc.tensor.matmul(
                            ps[:],
                            owt_sb[:, f, oc * 128:(oc + 1) * 128],
                            on_sb[:, f, :],
                            start=(f == 0), stop=(f == 7))
                    # h = x + attn_out (bf16)
                    nc.vector.tensor_tensor(h_sb[:, oc, :], xc_sb[:, oc, :],
                                            ps[:], OP.add)

        # ---------------- Phase E: MLP (fc1 -> silu -> fc2) ----------------
        # inner dim processed in 2 halves of 2048 to bound SBUF
        for half in range(2):
            with tc.tile_pool(name="ewt", bufs=1) as ewt, \
                 tc.tile_pool(name="ewk", bufs=3) as ewk, \
                 tc.tile_pool(name="zpool", bufs=1) as zpool, \
                 tc.tile_pool(name="epsum", bufs=3, space="PSUM") as epsum:
                fc1t_sb = ewt.tile([128, 8, 2048], BF16, tag="fc1t",
                                   name=f"fc1t_{half}")
                nc.sync.dma_start(
                    fc1t_sb[:],
                    fc1t.ap()[:, half * 2048:(half + 1) * 2048].rearrange(
                        "(c p) m -> p c m", p=128))
                fc2t_sb = ewt.tile([128, 16, H], BF16, tag="fc2t",
                                   name=f"fc2t_{half}")
                nc.sync.dma_start(
                    fc2t_sb[:],
                    fc2t.ap()[half * 2048:(half + 1) * 2048, :].rearrange(
                        "(c p) m -> p c m", p=128))
                z_sb = zpool.tile([128, 16, Q], BF16, tag="z", name=f"z{half}")
                for ic in range(16):
                    icg = half * 16 + ic
                    ps = epsum.tile([128, Q], F32, tag="z1ps", name=f"z1_{half}_{ic}")
                    for f in range(8):
                        nc.tensor.matmul(
                            ps[:],
                            fc1t_sb[:, f, ic * 128:(ic + 1) * 128],
                            h_sb[:, f, :],
                            start=(f == 0), stop=(f == 7))
                    sg = ewk.tile([128, Q], F32, tag="sg", name=f"sg{half}_{ic}")
                    nc.scalar.activation(sg[:], ps[:], AF.Sigmoid,
                                         bias=fc1b_sb[:, icg:icg + 1])
                    nc.vector.scalar_tensor_tensor(
                        z_sb[:, ic, :], ps[:], fc1b_sb[:, icg:icg + 1], sg[:],
                        OP.add, OP.mult)
                for oc in range(8):
                    ps = epsum.tile([128, Q], F32, tag="z2ps", name=f"z2_{half}_{oc}")
                    for ic in range(16):
                        nc.tensor.matmul(
                            ps[:],
                            fc2t_sb[:, ic, oc * 128:(oc + 1) * 128],
                            z_sb[:, ic, :],
                            start=(ic == 0), stop=(ic == 15))
                    if half == 0:
                        # mixed = fc2_half0 + fc2_b
                        nc.scalar.activation(mixed_sb[:, oc, :], ps[:], AF.Identity,
                                             bias=fc2b_sb[:, oc:oc + 1])
                    else:
                        nc.vector.tensor_tensor(
                            mixed_sb[:, oc, :], mixed_sb[:, oc, :], ps[:], OP.add)

        # ---------------- Phase F: RMSNorm -> upd -> shortcut ----------------
        with tc.tile_pool(name="fpool", bufs=1) as fpool, \
             tc.tile_pool(name="fpsum", bufs=3, space="PSUM") as fpsum, \
             tc.tile_pool(name="sqpsum", bufs=1, space="PSUM") as sqpsum:
            # sum of squares over feature dim (partitions x chunks) via fp32 PE
            msq_sb = fpool.tile([128, 8, Q], F32, tag="msq")
            nc.scalar.activation(msq_sb[:], mixed_sb[:], AF.Square)
            ssq = sqpsum.tile([1, Q], F32, tag="ssq")
            for c in range(8):
                nc.tensor.matmul(
                    ssq[:], ones1_sb[:], msq_sb[:, c, :],
                    start=(c == 0), stop=(c == 7))
            srow = fpool.tile([1, Q], F32, tag="srow")
            nc.scalar.activation(srow[:], ssq[:], AF.Sqrt,
                                 bias=eps_sb[:], scale=1.0 / H)
            rrow = fpool.tile([1, Q], F32, tag="rrow")
            nc.vector.reciprocal(rrow[:], srow[:])
            rb = fpool.tile([128, Q], F32, tag="rb")
            nc.gpsimd.partition_broadcast(rb[:], rrow[:])
            # precond_pre = mixed * rms * norm_w   (bf16, feeds upd matmul)
            pp_sb = fpool.tile([128, 8, Q], BF16, tag="pp")
            for c in range(8):
                nc.vector.scalar_tensor_tensor(
                    pp_sb[:, c, :], mixed_sb[:, c, :], normw_sb[:, c:c + 1], rb[:],
                    OP.mult, OP.mult)
            # precond = upd_w @ pp + upd_b ; s = mixed + precond (bf16)
            with tc.tile_pool(name="wres", bufs=1) as wres:
                updt_sb = wres.tile([128, 8, H], BF16, tag="updt")
                nc.sync.dma_start(updt_sb[:], updt.ap().rearrange("(c p) m -> p c m", p=128))
                sct_sb = wres.tile([128, 8, H], BF16, tag="sct")
                nc.sync.dma_start(sct_sb[:], sct.ap().rearrange("(c p) m -> p c m", p=128))
                prec_sb = fpool.tile([128, 8, Q], F32, tag="prec")
                for oc in range(8):
                    ps = fpsum.tile([128, Q], F32, tag="updps")
                    for f in range(8):
                        nc.tensor.matmul(ps[:], updt_sb[:, f, oc * 128:(oc + 1) * 128],
                                         pp_sb[:, f, :],
                                         start=(f == 0), stop=(f == 7))
                    nc.scalar.activation(prec_sb[:, oc, :], ps[:], AF.Identity,
                                         bias=updb_sb[:, oc:oc + 1])
                s_sb = fpool.tile([128, 8, Q], BF16, tag="s")
                for c in range(8):
                    nc.vector.tensor_tensor(
                        s_sb[:, c, :], mixed_sb[:, c, :], prec_sb[:, c, :], OP.add)
                # updated = xc + sc_w @ s + sc_b
                out_sb = fpool.tile([128, 8, Q], F32, tag="outsb")
                for oc in range(8):
                    ps = fpsum.tile([128, Q], F32, tag="scps")
                    for f in range(8):
                        nc.tensor.matmul(ps[:], sct_sb[:, f, oc * 128:(oc + 1) * 128],
                                         s_sb[:, f, :],
                                         start=(f == 0), stop=(f == 7))
                    nc.vector.scalar_tensor_tensor(
                        out_sb[:, oc, :], ps[:], scb_sb[:, oc:oc + 1], xc_sb[:, oc, :],
                        OP.add, OP.add)
                nc.sync.dma_start(out.ap().rearrange("(c p) t -> p c t", p=128), out_sb[:])


# ---------------------------------------------------------------------------
# Host-side sharding / gather
# ---------------------------------------------------------------------------

def _eo_cols(w_qk_head):
    """Permute head rows [64, H] -> [e(32) | o(32)] order."""
    return np.concatenate([w_qk_head[0::2], w_qk_head[1::2]], axis=0)


def make_in_maps(x, qkv_w, out_w, fc1_w, fc1_b, fc2_w, fc2_b, norm_w,
                 upd_w, upd_b, sc_w, sc_b):
    x = np.asarray(x, np.float32)
    qkv_w = np.asarray(qkv_w, np.float32)
    out_w = np.asarray(out_w, np.float32)
    fc1_w = np.asarray(fc1_w, np.float32)
    fc2_w = np.asarray(fc2_w, np.float32)
    upd_w = np.asarray(upd_w, np.float32)
    sc_w = np.asarray(sc_w, np.float32)
    qw = qkv_w[0:H].reshape(HEADS, HD, H)
    kw = qkv_w[H:2 * H].reshape(HEADS, HD, H)
    vw = qkv_w[2 * H:3 * H].reshape(HEADS, HD, H)

    # rope tables [128, S]: row p -> freq index p % 32
    d = np.arange(0, HD, 2, dtype=np.float32) / HD
    inv_freq = 1.0 / (ROPE_THETA ** d)                      # [32]
    tpos = np.arange(S, dtype=np.float32)
    freqs = tpos[None, :] * inv_freq[:, None]               # [32, S]
    cosf = np.tile(np.cos(freqs), (4, 1)).astype(NP_BF16)
    sinf = np.tile(np.sin(freqs), (4, 1)).astype(NP_BF16)

    def b8(v):
        return np.ascontiguousarray(np.asarray(v, np.float32).reshape(8, 128).T)

    def bf(a):
        return np.ascontiguousarray(np.asarray(a).astype(NP_BF16))

    # q,k weights with RoPE-ready layout: 8 q-pair col blocks, 8 k-pair blocks
    cols = []
    for w, scale in ((qw, 0.125), (kw, 1.0)):
        for j in range(NP):
            hA, hB = 2 * j, 2 * j + 1
            blk = np.concatenate([_eo_cols(w[hA]), _eo_cols(w[hB])], axis=0) * scale
            cols.append(blk)  # [128, H]
    qkt = bf(np.concatenate(cols, axis=0).T)                 # [H, 2048]
    vwt = bf(np.concatenate([vw[h] for h in range(HEADS)], axis=0).T)  # [H, H]
    owt_ = bf(out_w.T)
    fc1t_ = bf(fc1_w.T)
    fc1b_ = np.ascontiguousarray(
        np.asarray(fc1_b, np.float32).reshape(32, 128).T)
    fc2t_ = bf(fc2_w.T)
    updt_ = bf(upd_w.T)
    sct_ = bf(sc_w.T)

    in_maps = []
    for c in range(NCORES):
        g, r = c // GSZ, c % GSZ
        # token permutation: own query chunk first, remaining tokens after
        perm = np.r_[np.arange(Q * r, Q * (r + 1)),
                     np.arange(0, Q * r), np.arange(Q * (r + 1), S)]
        in_maps.append({
            "xt": bf(x[g].T[:, perm]),
            "xc": np.ascontiguousarray(x[g][Q * r:Q * (r + 1), :].T),
            "qkt": qkt,
            "vwt": vwt,
            "owt": owt_,
            "fc1t": fc1t_,
            "fc1b": fc1b_,
            "fc2t": fc2t_,
            "fc2b": b8(fc2_b),
            "updt": updt_,
            "updb": b8(upd_b),
            "sct": sct_,
            "scb": b8(sc_b),
            "normw": b8(norm_w),
            "cosf": np.ascontiguousarray(cosf[:, perm]),
            "sinf": np.ascontiguousarray(sinf[:, perm]),
        })
    return in_maps


def run(inputs, trace=False, reps=1, **kw):
    nc = build_program(reps)
    in_maps = make_in_maps(**inputs)
    res = run_bass_kernel_spmd(nc, in_maps, list(range(NCORES)), trace=trace, **kw)
    outs = np.empty((B, S, H), np.float32)
    for c in range(NCORES):
        g, r = c // GSZ, c % GSZ
        outs[g, Q * r:Q * (r + 1), :] = res.results[c]["out"].T
    return outs, res


def kernel(**inputs):
    outs, _ = run(inputs)
    return outs
